# revision 36
# baseline (speedup 1.0000x reference)
"""Trainium2 Bass kernel for nn_BeBertEmbedding (self-contained).

Math: the reference's semantic_embed(ids, W, b, pad=0) is
    where(ids==0, take(W.T, ids) + b, zeros)
so the only table row that survives is W[:, 0], and the whole module is
    out[b,s,:] = pe[s,:] + (ids[b,s]==0)*(W_tok[:,0]+b_tok)
                         + (seg[b,s]==0)*(W_seg[:,0]+b_seg)

Sharding: sequence-parallel across 8 cores (256 positions/core, all 16
batches; each core writes a disjoint [16, 256, 768] slice, no collectives).

Per core the device program is raw Bass (no Tile — avoids the kernel-tail
drain/barrier):
  * one packed const tensor: [c_seg | seg-masks | (c_tok | tok-masks) |
    pe tile0 | pe tile1], loaded in two DMAs (the first covers everything
    the t=0 tiles need, so compute starts before pe tile1 lands),
  * 32 output tiles [128 tokens, 768]: one fused DVE scalar_tensor_tensor
    (c_seg * m2[p]) + pe each (plus a second op for the rare tiles with a
    zero token id), through NSLOT SBUF slots,
  * out-DMAs issued alternately from the SP and ACT sequencers — TRN2's two
    physical HW-DGE rings — keeping the stream HBM-write-bandwidth-bound,
  * per-slot semaphores (DMAs on different HW queues complete out of order,
    a single counting sem cannot tell WHICH slot freed).

Cost-model estimate 33.2us/core; streams in isolation: DVE ~29.4us, HWDGE
writes 24.0us (model; real HBM is ~358GB/s per core SHARED by reads and
writes, so the real floor is (12MB out + 0.8MB in)/358GB/s ~= 36us — this
kernel sits on that roofline).  Rejected variants kept behind env flags:
BASS_KERNEL_POOL_SPLIT (walrus: no TensorScalarPtr on Pool),
BASS_KERNEL_DEV_BCAST (on-device c_seg broadcast saves 0.4MB HBM read but
the cvec DMA + gpsimd ucode library reload + broadcast chain costs more
head than it saves).  An indirect-DMA scatter variant (2 DVE ops + 2 SWDGE
scatters, see sim_scatter.py in the dev tree) sims at 10.6us but that is a
cost-model artifact — it does not charge SWDGE data movement; real SWDGE
queue->engine spread is unknown, so it is not shipped.
"""

import contextlib
import os
import sys

import numpy as np

try:
    from concourse import bacc, bass, mybir, tile
    from concourse.bass_utils import run_bass_kernel_spmd
except ImportError:
    for _p in ("/opt/trn_rl_repo", "/root/.axon_site/_ro/trn_rl_repo"):
        if os.path.isdir(_p) and _p not in sys.path:
            sys.path.insert(0, _p)
            break
    from concourse import bacc, bass, mybir, tile
    from concourse.bass_utils import run_bass_kernel_spmd

N_CORES = 8
B, SEQ, D = 16, 2048, 768
S_SH = SEQ // N_CORES        # 256 sequence positions per core
P = 128                      # partitions
T_TILES = S_SH // P          # 2 seq tiles per core
J = B * T_TILES              # 32 output tiles per core
NSLOT = 16                   # SBUF output slots

_F32 = mybir.dt.float32

_prog_cache: dict = {}
LAST_RESULTS = None          # BassKernelResults of the most recent run


# ---------------------------------------------------------------- raw bass —

def _layout(tok_cols, dev_bcast=False):
    """Column offsets inside the packed per-core const tensor [128, C].
    With dev_bcast the c-vectors are NOT in this tensor (they arrive as a
    [1, D]-per-vector tensor and are partition-broadcast on device)."""
    if dev_bcast:
        M2_OFF = 0
        off = M2_OFF + J
        if tok_cols:
            M1_OFF = off
            off = M1_OFF + J
        else:
            M1_OFF = None
        CSEG_OFF = CTOK_OFF = None
        PE_OFF = off
        C = PE_OFF + T_TILES * D
        A_COLS = PE_OFF + D
        return CSEG_OFF, M2_OFF, CTOK_OFF, M1_OFF, PE_OFF, C, A_COLS
    CSEG_OFF = 0
    M2_OFF = CSEG_OFF + D
    off = M2_OFF + J
    if tok_cols:
        CTOK_OFF = off
        M1_OFF = CTOK_OFF + D
        off = M1_OFF + J
    else:
        CTOK_OFF = M1_OFF = None
    PE_OFF = off
    C = PE_OFF + T_TILES * D
    A_COLS = PE_OFF + D          # DMA-A: everything + pe tile 0
    return CSEG_OFF, M2_OFF, CTOK_OFF, M1_OFF, PE_OFF, C, A_COLS


def _order(i):
    """Compute-order index i -> output tile j: all t=0 tiles first (they
    only need the first const DMA), then t=1 tiles."""
    return 2 * i if i < J // 2 else 2 * (i - J // 2) + 1


def _build_raw(
    tok_cols: frozenset, pool_split: bool = False, dev_bcast: bool = False
) -> "bass.Bass":
    CSEG_OFF, M2_OFF, CTOK_OFF, M1_OFF, PE_OFF, C, A_COLS = _layout(
        tok_cols, dev_bcast
    )
    assert not (pool_split and dev_bcast)
    NV = 2 if tok_cols else 1    # number of broadcast c-vectors
    nc = bass.Bass("TRN2")
    const_d = nc.dram_tensor("consts", [P, C], _F32, kind="ExternalInput")
    if dev_bcast:
        cvec_d = nc.dram_tensor("cvec", [1, NV * D], _F32, kind="ExternalInput")
    out_d = nc.dram_tensor("out", [B * S_SH, D], _F32, kind="ExternalOutput")
    mult, add = mybir.AluOpType.mult, mybir.AluOpType.add

    # Which compute order-indices run on GPSIMD (Pool) instead of DVE.
    # i%4==3 keeps slot ownership disjoint (slots 3,7 are pool-exclusive, so
    # same-slot WAR chains never cross engines).  Tok tiles stay on DVE.
    if pool_split:
        on_pool = [i % 4 == 3 and _order(i) not in tok_cols for i in range(J)]
    else:
        on_pool = [False] * J
    # per-engine completion counts up to and including index i
    dve_cnt, pool_cnt, dc, pc = [], [], 0, 0
    for i in range(J):
        if on_pool[i]:
            pc += 1
        else:
            dc += 1
        dve_cnt.append(dc)
        pool_cnt.append(pc)

    with contextlib.ExitStack() as stack:
        c_t = stack.enter_context(nc.sbuf_tensor([P, C], _F32))
        if dev_bcast:
            cw = stack.enter_context(nc.sbuf_tensor([P, NV * D], _F32))
            cv_sem = stack.enter_context(nc.semaphore("cv_sem"))
            bc_sem = stack.enter_context(nc.semaphore("bc_sem"))
        obuf = stack.enter_context(nc.sbuf_tensor([P, NSLOT * D], _F32))
        a_sem = stack.enter_context(nc.semaphore("a_sem"))
        b_sem = stack.enter_context(nc.semaphore("b_sem"))
        v_sem = stack.enter_context(nc.semaphore("v_sem"))
        p_sem = stack.enter_context(nc.semaphore("p_sem"))
        s_sems = [
            stack.enter_context(nc.semaphore(f"slot_sem{k}")) for k in range(NSLOT)
        ]
        # DVE's pipeline is deep: the tok tile's second STT reads the first's
        # output on the same engine and needs an explicit retire guard.
        t_sem = stack.enter_context(nc.semaphore("tok_sem"))
        block = stack.enter_context(nc.Block())

        def issue_out_dmas(eng, parity):
            for i in range(J):
                if i % 2 != parity:
                    continue
                j = _order(i)
                s = i % NSLOT
                done = (p_sem, pool_cnt[i]) if on_pool[i] else (v_sem, dve_cnt[i])
                # wait attached inline: TRN2 allows exactly one wait per
                # instruction, and this saves a sequencer dispatch slot
                eng.dma_start(
                    out_d[j * P:(j + 1) * P, :], obuf[:, s * D:(s + 1) * D]
                )._wait_ge(*done).then_inc(s_sems[s], 16)

        # Head: the first compute op needs masks + cseg + pe tile 0.  Those
        # load as THREE parallel-ish DMAs: [masks|cseg] then pe0-low-half on
        # the SP ring, pe0-high-half on the ACT ring — halving the largest
        # serial transfer in front of the first op.  pe1 (only needed from
        # i=J/2) follows on SP.
        PE0 = PE_OFF
        PE0M = PE_OFF + D // 2
        @block.sync
        def _(sync):
            if dev_bcast:
                sync.dma_start(cw[0:1, :], cvec_d[:]).then_inc(cv_sem, 16)
            sync.dma_start(c_t[:, :PE0], const_d[:, :PE0]).then_inc(a_sem, 16)
            sync.dma_start(c_t[:, PE0:PE0M], const_d[:, PE0:PE0M]).then_inc(a_sem, 16)
            sync.dma_start(c_t[:, A_COLS:], const_d[:, A_COLS:]).then_inc(b_sem, 16)
            issue_out_dmas(sync, 0)

        @block.scalar
        def _(scalar):
            scalar.dma_start(
                c_t[:, PE0M:A_COLS], const_d[:, PE0M:A_COLS]
            ).then_inc(a_sem, 16)
            issue_out_dmas(scalar, 1)

        if dev_bcast:
            @block.gpsimd
            def _(gpsimd):
                from concourse import library_config
                # PartitionBroadcast lives in the attnmlp/attn/mlp/proxy
                # gpsimd ucode libraries, not the default one
                nc.gpsimd.load_library(library_config.attnmlp)
                gpsimd.wait_ge(cv_sem, 16)
                for v in range(NV):
                    nc.gpsimd.partition_broadcast(
                        cw[:, v * D:(v + 1) * D], cw[0:1, v * D:(v + 1) * D]
                    ).then_inc(bc_sem, 1)

        def compute(eng_handle, eng_obj, my_flag, done_sem):
            if dev_bcast:
                cseg = cw[:, 0:D]
            else:
                cseg = c_t[:, CSEG_OFF:CSEG_OFF + D]
            n_tok = 0
            waited_b = False
            eng_handle.wait_ge(a_sem, 48)   # all three pieces of the A-load
            if dev_bcast:
                eng_handle.wait_ge(bc_sem, NV)
            for i in range(J):
                if on_pool[i] != my_flag:
                    continue
                j = _order(i)
                t = j % T_TILES
                s = i % NSLOT
                if t == 1 and not waited_b:
                    eng_handle.wait_ge(b_sem, 16)
                    waited_b = True
                # slot-reuse wait attached inline on the STT (one wait max
                # per instruction; saves a sequencer dispatch slot)
                slot_wait = (
                    (s_sems[s], 16 * (i // NSLOT)) if i >= NSLOT else None
                )
                o_sl = obuf[:, s * D:(s + 1) * D]
                pe_sl = c_t[:, PE_OFF + t * D:PE_OFF + (t + 1) * D]
                m2_col = c_t[:, M2_OFF + j:M2_OFF + j + 1]
                if j in tok_cols:
                    m1_col = c_t[:, M1_OFF + j:M1_OFF + j + 1]
                    if dev_bcast:
                        ctok = cw[:, D:2 * D]
                    else:
                        ctok = c_t[:, CTOK_OFF:CTOK_OFF + D]
                    # acc = m1*c_tok + pe, then m2*c_seg + acc: the same fp
                    # add order as the reference's (tok + pe) + seg.
                    eng_obj.scalar_tensor_tensor(
                        o_sl, ctok, m1_col, pe_sl, op0=mult, op1=add,
                    )._maybe_wait_ge(slot_wait).then_inc(t_sem, 1)
                    n_tok += 1
                    eng_obj.scalar_tensor_tensor(
                        o_sl, cseg, m2_col, o_sl, op0=mult, op1=add,
                    )._wait_ge(t_sem, n_tok).then_inc(done_sem, 1)
                else:
                    eng_obj.scalar_tensor_tensor(
                        o_sl, cseg, m2_col, pe_sl, op0=mult, op1=add,
                    )._maybe_wait_ge(slot_wait).then_inc(done_sem, 1)

        @block.vector
        def _(vector):
            compute(vector, nc.vector, False, v_sem)

        if pool_split:
            @block.gpsimd
            def _(gpsimd):
                compute(gpsimd, nc.gpsimd, True, p_sem)

    nc.finalize()
    return nc


def _prepare_raw(inputs: dict, dev_bcast: bool = False):
    ids = np.asarray(inputs["input_ids"])
    seg = np.asarray(inputs["segment_label"])
    W_tok = np.asarray(inputs["W_tok"], dtype=np.float32)
    b_tok = np.asarray(inputs["b_tok"], dtype=np.float32)
    W_seg = np.asarray(inputs["W_seg"], dtype=np.float32)
    b_seg = np.asarray(inputs["b_seg"], dtype=np.float32)
    pe = np.asarray(inputs["pe"], dtype=np.float32).reshape(SEQ, D)

    c_tok = (W_tok[:, 0] + b_tok).astype(np.float32)
    c_seg = (W_seg[:, 0] + b_seg).astype(np.float32)
    m1_full = (ids == 0).astype(np.float32)
    m2_full = (seg == 0).astype(np.float32)

    per_core = []
    tok_cols = set()
    for c in range(N_CORES):
        sl = slice(c * S_SH, (c + 1) * S_SH)
        # [B, S_SH] -> [P, J] with column j = b*T_TILES + t, partition p
        m1 = m1_full[:, sl].reshape(B, T_TILES, P).transpose(2, 0, 1).reshape(P, J)
        m2 = m2_full[:, sl].reshape(B, T_TILES, P).transpose(2, 0, 1).reshape(P, J)
        pe_sl = pe[sl].reshape(T_TILES, P, D).transpose(1, 0, 2).reshape(P, T_TILES * D)
        tok_cols.update(np.nonzero(m1.any(axis=0))[0].tolist())
        per_core.append((pe_sl, m1, m2))

    tok_cols = frozenset(tok_cols)
    CSEG_OFF, M2_OFF, CTOK_OFF, M1_OFF, PE_OFF, C, _ = _layout(tok_cols, dev_bcast)
    NV = 2 if tok_cols else 1
    cvec = None
    if dev_bcast:
        cvec = np.empty((1, NV * D), dtype=np.float32)
        cvec[0, :D] = c_seg
        if tok_cols:
            cvec[0, D:] = c_tok
    in_maps = []
    for pe_sl, m1, m2 in per_core:
        consts = np.empty((P, C), dtype=np.float32)
        consts[:, M2_OFF:M2_OFF + J] = m2
        if tok_cols:
            consts[:, M1_OFF:M1_OFF + J] = m1
        if not dev_bcast:
            consts[:, CSEG_OFF:CSEG_OFF + D] = c_seg
            if tok_cols:
                consts[:, CTOK_OFF:CTOK_OFF + D] = c_tok
        consts[:, PE_OFF:PE_OFF + T_TILES * D] = pe_sl
        m = {"consts": consts}
        if dev_bcast:
            m["cvec"] = cvec
        in_maps.append(m)
    return in_maps, tok_cols


# -------------------------------------------------- tile variant (fallback) —

TPE_OFF = 0
TCSEG_OFF = T_TILES * D
TM2_OFF = TCSEG_OFF + D
TC_COMMON = TM2_OFF + J
TCTOK_OFF = TC_COMMON
TM1_OFF = TCTOK_OFF + D
TC_FULL = TM1_OFF + J


def _build_tile(tok_cols: frozenset) -> "bass.Bass":
    C = TC_FULL if tok_cols else TC_COMMON
    # Bacc (not plain Bass): its compile() splits multi-semaphore waits into
    # event semaphores — TRN2 allows at most one inline wait per instruction.
    nc = bacc.Bacc("TRN2", target_bir_lowering=False)
    const_d = nc.dram_tensor("consts", [P, C], _F32, kind="ExternalInput")
    out_d = nc.dram_tensor("out", [B * S_SH, D], _F32, kind="ExternalOutput")
    mult, add = mybir.AluOpType.mult, mybir.AluOpType.add

    with tile.TileContext(nc) as tc:
        with (
            tc.tile_pool(name="const", bufs=1) as cpool,
            tc.tile_pool(name="outp", bufs=8) as opool,
        ):
            c_t = cpool.tile([P, C], _F32)
            nc.sync.dma_start(c_t[:], const_d[:])
            cseg_t = c_t[:, TCSEG_OFF:TCSEG_OFF + D]
            ctok_t = c_t[:, TCTOK_OFF:TCTOK_OFF + D] if tok_cols else None

            for b in range(B):
                for t in range(T_TILES):
                    j = b * T_TILES + t
                    o = opool.tile([P, D], _F32, tag="out")
                    pe_slice = c_t[:, TPE_OFF + t * D:TPE_OFF + (t + 1) * D]
                    m2_col = c_t[:, TM2_OFF + j:TM2_OFF + j + 1]
                    if j in tok_cols:
                        m1_col = c_t[:, TM1_OFF + j:TM1_OFF + j + 1]
                        nc.vector.scalar_tensor_tensor(
                            o[:], ctok_t, m1_col, pe_slice, op0=mult, op1=add,
                        )
                        nc.vector.scalar_tensor_tensor(
                            o[:], cseg_t, m2_col, o[:], op0=mult, op1=add,
                        )
                    else:
                        nc.vector.scalar_tensor_tensor(
                            o[:], cseg_t, m2_col, pe_slice, op0=mult, op1=add,
                        )
                    nc.sync.dma_start(out_d[j * P:(j + 1) * P, :], o[:])
    nc.finalize()
    return nc


def _prepare_tile(inputs: dict):
    ids = np.asarray(inputs["input_ids"])
    seg = np.asarray(inputs["segment_label"])
    W_tok = np.asarray(inputs["W_tok"], dtype=np.float32)
    b_tok = np.asarray(inputs["b_tok"], dtype=np.float32)
    W_seg = np.asarray(inputs["W_seg"], dtype=np.float32)
    b_seg = np.asarray(inputs["b_seg"], dtype=np.float32)
    pe = np.asarray(inputs["pe"], dtype=np.float32).reshape(SEQ, D)

    c_tok = (W_tok[:, 0] + b_tok).astype(np.float32)
    c_seg = (W_seg[:, 0] + b_seg).astype(np.float32)
    m1_full = (ids == 0).astype(np.float32)
    m2_full = (seg == 0).astype(np.float32)

    per_core = []
    tok_cols = set()
    for c in range(N_CORES):
        sl = slice(c * S_SH, (c + 1) * S_SH)
        m1 = m1_full[:, sl].reshape(B, T_TILES, P).transpose(2, 0, 1).reshape(P, J)
        m2 = m2_full[:, sl].reshape(B, T_TILES, P).transpose(2, 0, 1).reshape(P, J)
        pe_sl = pe[sl].reshape(T_TILES, P, D).transpose(1, 0, 2).reshape(P, T_TILES * D)
        tok_cols.update(np.nonzero(m1.any(axis=0))[0].tolist())
        per_core.append((pe_sl, m1, m2))

    need_tok = bool(tok_cols)
    C = TC_FULL if need_tok else TC_COMMON
    in_maps = []
    for pe_sl, m1, m2 in per_core:
        consts = np.empty((P, C), dtype=np.float32)
        consts[:, TPE_OFF:TPE_OFF + T_TILES * D] = pe_sl
        consts[:, TCSEG_OFF:TCSEG_OFF + D] = c_seg
        consts[:, TM2_OFF:TM2_OFF + J] = m2
        if need_tok:
            consts[:, TCTOK_OFF:TCTOK_OFF + D] = c_tok
            consts[:, TM1_OFF:TM1_OFF + J] = m1
        in_maps.append({"consts": consts})
    return in_maps, frozenset(tok_cols)


# ------------------------------------------------------------------- entry —

def kernel(**inputs) -> np.ndarray:
    global LAST_RESULTS
    impl = os.environ.get("BASS_KERNEL_IMPL", "raw")
    if impl == "raw":
        # NOTE: pool_split compiles in CoreSim but walrus rejects
        # TensorScalarPtr on the Pool engine (NCC_IXCG966) — keep off.
        pool_split = bool(int(os.environ.get("BASS_KERNEL_POOL_SPLIT", "0")))
        dev_bcast = bool(int(os.environ.get("BASS_KERNEL_DEV_BCAST", "0")))
        in_maps, tok_cols = _prepare_raw(inputs, dev_bcast=dev_bcast)
        key = ("raw", pool_split, dev_bcast, tok_cols)
        def builder(tc):
            return _build_raw(tc, pool_split=pool_split, dev_bcast=dev_bcast)
    else:
        in_maps, tok_cols = _prepare_tile(inputs)
        key = ("tile", tok_cols)
        builder = _build_tile
    # SPMD: one program for all cores; the tok op is emitted for any column
    # that needs it on any core (a zero mask column makes it the identity).
    if key not in _prog_cache:
        _prog_cache[key] = builder(tok_cols)
    nc = _prog_cache[key]

    trace = bool(int(os.environ.get("BASS_KERNEL_TRACE", "0")))
    try:
        res = run_bass_kernel_spmd(
            nc, in_maps, list(range(N_CORES)), trace=trace,
            trace_cores=list(range(N_CORES)) if trace else None,
        )
    except ModuleNotFoundError:
        # axon builds without the NTFF profile hook (antenv.axon_hooks)
        # crash when tracing is requested (e.g. BASS_TRACE=1 in the env);
        # degrade to an untraced run rather than failing the kernel.
        os.environ["BASS_NEVER_TRACE"] = "1"
        res = run_bass_kernel_spmd(nc, in_maps, list(range(N_CORES)), trace=False)
    LAST_RESULTS = res

    out = np.empty((B, SEQ, D), dtype=np.float32)
    for c in range(N_CORES):
        out[:, c * S_SH:(c + 1) * S_SH, :] = (
            np.asarray(res.results[c]["out"]).reshape(B, S_SH, D)
        )
    return out
